# revision 1
# baseline (speedup 1.0000x reference)
"""Trainium2 Bass kernel for nn_NoiseConv1d (channel-wise 6-bit quantize + 1x1 conv).

Math (per batch b, position n, with QL=63):
    cmax/cmin = max/min over channels c of x[b,:,n]
    s = (cmax-cmin)/QL ; q = round((x-cmin)/s)   (q in [0,63], exact in fp16)
    out[o,n] = s[n] * sum_c W[o,c] q[c,n] + cmin[n]*Wsum[o] + bias[o]

Kernel strategy (one batch per NeuronCore, 8 cores data-parallel), mode "petr":
    - load x [512c, 4096n] natural layout (fp32)
    - PE-transpose 128x128 blocks -> PSUM, ACT-evac to SBUF xT [128n, 512c]
      (psum recycles fast; SBUF source gives DVE 2x mode)
    - DVE tensor_scalar+accum: per-n max and -min over channels (2x mode)
    - ACT quant: y+1536 = r*x + (-cmin*r + 1536) per-partition scale/bias fma;
      the fp32->fp16 output conversion RNE-rounds to integer+1536 for free
      (fp16 ulp == 1 in [1024, 2048))
    - DVE tensor_scalar: qs = (q16 - 1536) * s -> fp16 (exact q, one rounding)
    - cmin hi/lo fp16 split -> tiny [128,2] PE transpose; offset correction
      becomes a K=2 extra matmul with lhsT rows [Wsum_hi-ish, Wsum]
    - PE-transpose qs back to [c, n] (fp16 transpose-mode, 1 cyc/row) + DVE
      evac. (The xbar DMA-transpose path was 1.6x slower end-to-end: ~3us
      per call and it serializes against copy DMAs.)
    - PE matmul fp16 (psum fp32): 4 k-tiles + K=2 offset rows per [o,512n]
      psum tile; ACT evac folds the bias add; DMA out.
    Measured ~92.5 us/core on trn2 (8 cores parallel => full batch B=8);
    rel err vs reference ~1.4e-3.
"""

import sys

sys.path.insert(0, "/opt/trn_rl_repo")

import numpy as np

B, C, COUT, N = 8, 512, 512, 4096
QL = 63.0
MAGIC = float(np.float32(12582912.0))  # 1.5 * 2**23 : fp32 RNE round-to-int magic
MAGIC16 = 1536.0  # 1.5 * 2**10 : fp16 RNE round-to-int magic (ulp=1 in [1024,2048))

KT = C // 128  # 4 c-tiles
MT = COUT // 128  # 4 o-tiles
NT = N // 128  # 32 n-tiles
NCH = N // 512  # 8 n-chunks (psum free dim 512)

_cache = {}


def _build_bass(loop_n=0, mode="full"):
    from contextlib import ExitStack

    import concourse.bass as bass
    import concourse.mybir as mybir
    from concourse import bacc
    from concourse.bass import ds, ts
    from concourse.masks import make_identity
    from concourse.tile import TileContext

    f32 = mybir.dt.float32
    f16 = mybir.dt.float16
    AX = mybir.AxisListType
    OP = mybir.AluOpType
    AF = mybir.ActivationFunctionType

    nc = bacc.Bacc(None, target_bir_lowering=False)
    xb = nc.dram_tensor("xb", [C, N], f32, kind="ExternalInput")
    wt = nc.dram_tensor("wt", [C, COUT], f16, kind="ExternalInput")  # W^T [c,o]
    wr = nc.dram_tensor("wr", [2, COUT], f16, kind="ExternalInput")  # [Whi,Whi]
    bv = nc.dram_tensor("bv", [128, MT], f32, kind="ExternalInput")  # bias packed
    out = nc.dram_tensor("out", [COUT, N], f32, kind="ExternalOutput")

    with ExitStack() as ctx:
        tc = ctx.enter_context(TileContext(nc))
        singles = ctx.enter_context(tc.tile_pool(name="singles", bufs=1))

        NQ = N // 4  # n-quarter size
        if "cf16" in mode:
            x_nat = [
                singles.tile([128, KT, NQ], f16, name=f"x{q}", tag=f"x{q}")
                for q in range(4)
            ]
        else:
            x_nat = [
                singles.tile([128, KT, NQ], f32, name=f"x{q}", tag=f"x{q}")
                for q in range(4)
            ]
        if "f16x" in mode:
            # fp16 copy of x (natural layout) + its xbar-transposed form
            x_b = singles.tile([128, KT, N], f16)
            # per quarter: xtb[q][n%128, k, t_local, c%128] = x[c, n]
            xtb = [
                singles.tile([128, KT, NT // 4, 128], f16, name=f"xtb{q}", tag=f"xtb{q}")
                for q in range(4)
            ]
        wt_sb = singles.tile([128, KT, COUT], f16)
        wr_sb = singles.tile([2, COUT], f16)
        bv_sb = singles.tile([128, MT], f32)
        ident = singles.tile([128, 128], f32)
        ident16 = singles.tile([128, 128], f16)

        make_identity(nc, ident)
        make_identity(nc, ident16)
        nc.sync.dma_start(out=wt_sb, in_=wt.rearrange("(k p) o -> p k o", p=128))
        nc.sync.dma_start(out=wr_sb, in_=wr[:, :])
        nc.sync.dma_start(out=bv_sb, in_=bv[:, :])
        xq = xb.rearrange("(k p) n -> p k n", p=128)

        ps_tr = ctx.enter_context(tc.tile_pool(name="ps_tr", bufs=3, space="PSUM"))
        ps_qt = ctx.enter_context(tc.tile_pool(name="ps_qt", bufs=2, space="PSUM"))
        ps_mm = ctx.enter_context(tc.tile_pool(name="ps_mm", bufs=3, space="PSUM"))
        stat = ctx.enter_context(tc.tile_pool(name="stat", bufs=12))
        qpool = ctx.enter_context(tc.tile_pool(name="qpool", bufs=6))
        jpool = ctx.enter_context(tc.tile_pool(name="jpool", bufs=4))
        opool = ctx.enter_context(tc.tile_pool(name="opool", bufs=4))
        qtpool = ctx.enter_context(tc.tile_pool(name="qtpool", bufs=3))

        def per_iter():
            for q in range(4):
                if "cf16" in mode:
                    # SWDGE cast-DMA: fp32 DRAM -> fp16 SBUF directly
                    nc.gpsimd.dma_start(out=x_nat[q], in_=xq[:, :, ds(q * NQ, NQ)])
                else:
                    nc.sync.dma_start(out=x_nat[q], in_=xq[:, :, ds(q * NQ, NQ)])
            if "f16x" in mode:
                for q in range(4):
                    nc.vector.tensor_copy(
                        out=x_b[:, :, ds(q * NQ, NQ)], in_=x_nat[q]
                    )
                    for k in range(KT):
                        nc.sync.dma_start_transpose(
                            out=xtb[q][:, k],
                            in_=x_b[:, k, ds(q * NQ, NQ)],
                        )
            if "dma" in mode:
                # DMA floor: just stream junk out at full volume
                for j in range(NCH):
                    for m in range(MT):
                        nc.sync.dma_start(
                            out=out[ts(m, 128), ds(512 * j, 512)],
                            in_=x_nat[0][:, 0, 0:512],
                        )
                return
            body_chunks()

        def body_chunks():
          for j in range(NCH):
            # qsT[c%128, i, c//128, n%128]
            qsT = qtpool.tile([128, 4, KT, 128], f16, tag="qsT")
            # cmin hi/lo rows for this chunk: cmr[row, i, n%128]
            cmr = qtpool.tile([2, 4, 128], f16, tag="cmr")
            hl = stat.tile([128, 2, 4], f16, tag="hl")  # hi/lo per tile col
            for i in range(4):
                t = 4 * j + i
                if "f16x" in mode:
                    # x already transposed to [n, c] fp16 by the xbar DMA
                    xts = xtb[t // 8][:, :, t % 8, :]
                else:
                    # ---- transpose x block column t into [n, c] psum tile ----
                    cf = "cf16" in mode
                    xt_ps = ps_tr.tile([128, 512], f16 if cf else f32, tag="xt")
                    xs = x_nat[t // 8]
                    tl = t % 8
                    for k in range(KT):
                        nc.tensor.transpose(
                            xt_ps[:, ts(k, 128)],
                            xs[:, k, ts(tl, 128)],
                            ident16 if cf else ident,
                        )
                    # evac psum -> sbuf right away so psum slots recycle fast
                    # and the DVE stats below run at higher perf mode
                    xts_t = qpool.tile([128, 512], f16 if cf else f32, tag="xts")
                    nc.scalar.copy(out=xts_t, in_=xt_ps)
                    xts = xts_t
                # ---- per-n stats over free dim via tensor_scalar accumulate ----
                mx = stat.tile([128, 1], f32, tag="mx")
                ng = stat.tile([128, 1], f32, tag="ng")  # -cmin
                junk = jpool.tile([128, 512], f16, tag="junk")
                nc.vector.tensor_scalar(
                    out=junk, in0=xts, scalar1=1.0, scalar2=None,
                    op0=OP.mult, op1=OP.max, accum_out=mx,
                )
                nc.vector.tensor_scalar(
                    out=junk, in0=xts, scalar1=-1.0, scalar2=None,
                    op0=OP.mult, op1=OP.max, accum_out=ng,
                )
                d = stat.tile([128, 1], f32, tag="d")
                s = stat.tile([128, 1], f32, tag="s")
                r = stat.tile([128, 1], f32, tag="r")
                Bt = stat.tile([128, 1], f32, tag="Bt")
                nc.vector.tensor_add(d, mx, ng)  # d = cmax - cmin
                nc.vector.tensor_scalar_mul(s, d, 1.0 / QL)  # s = scaling
                nc.vector.reciprocal(r, s)  # r = 1/s
                # B = -cmin*r + 1536 (fp16 magic; fp32 ulp@1536 keeps fraction)
                nc.vector.tensor_scalar(
                    out=Bt, in0=ng, scalar1=r, scalar2=MAGIC16,
                    op0=OP.mult, op1=OP.add,
                )
                if "noq" in mode:
                    continue
                # ---- quantize: y+1536 = r*x + B via ACT fma; the fp32->fp16
                # output conversion RNE-rounds to integer+1536 exactly ----
                qp = qpool.tile([128, 512], f16, tag="qp")
                nc.scalar.activation(
                    out=qp, in_=xts, func=AF.Identity, bias=Bt, scale=r
                )
                # ---- qs = (q16 - 1536) * s -> fp16 ----
                qs = qpool.tile([128, KT * 128], f16, tag="qs")
                nc.vector.tensor_scalar(
                    out=qs, in0=qp, scalar1=MAGIC16, scalar2=s,
                    op0=OP.subtract, op1=OP.mult,
                )
                # cmin hi/lo for this tile: hi = f16(cmin); lo ~= cmin - hi
                nc.gpsimd.tensor_scalar_mul(hl[:, 0, i : i + 1], ng, -1.0)
                nc.gpsimd.tensor_scalar(
                    out=hl[:, 1, i : i + 1], in0=hl[:, 0, i : i + 1],
                    scalar1=ng, scalar2=-1.0, op0=OP.add, op1=OP.mult,
                )
                # ---- transpose back to [c, n] layout ----
                if "petr" in mode:
                    # PE transpose (f16, 1 cyc/row) + DVE evac; avoids the
                    # xbar DMA-transpose path entirely (slow + serializes
                    # against copy DMAs)
                    qt_ps = ps_qt.tile([128, (KT + 1) * 128], f16, tag="qtp")
                    for b_ in range(KT):
                        nc.tensor.transpose(
                            qt_ps[:, ts(b_, 128)], qs[:, ts(b_, 128)], ident16
                        )
                    nc.tensor.transpose(
                        qt_ps[0:2, ts(KT, 128)], hl[:, :, i], ident16
                    )
                    nc.vector.tensor_copy(out=qsT[:, i], in_=qt_ps[:, 0 : KT * 128])
                    nc.vector.tensor_copy(
                        out=cmr[:, i], in_=qt_ps[0:2, ts(KT, 128)]
                    )
                else:
                    nc.sync.dma_start_transpose(out=qsT[:, i], in_=qs)

            # ---- matmul for this n-chunk ----
            if "nomm" in mode:
                continue
            for m in range(MT):
                ps = ps_mm.tile([128, 512], f32, tag="mm")
                for k in range(KT):
                    nc.tensor.matmul(
                        ps,
                        wt_sb[:, k, ts(m, 128)],
                        qsT[:, :, k, :],
                        start=(k == 0),
                        stop=False,
                    )
                nc.tensor.matmul(
                    ps,
                    wr_sb[:, ts(m, 128)],
                    cmr[:, :, :],
                    start=False,
                    stop=True,
                )
                ob = opool.tile([128, 512], f32, tag="ob")
                nc.scalar.activation(
                    out=ob, in_=ps, func=AF.Identity,
                    bias=bv_sb[:, m : m + 1], scale=1.0,
                )
                nc.sync.dma_start(
                    out=out[ts(m, 128), ds(512 * j, 512)], in_=ob
                )

        if loop_n:
            with tc.For_i(0, loop_n, 1):
                per_iter()
        else:
            per_iter()

    nc.compile()
    return nc


def _prep_weights(weight, bias):
    W = weight[:, :, 0].astype(np.float64)  # [o, c]
    wt = np.ascontiguousarray(W.T).astype(np.float16)  # [c, o]
    wsum = W.sum(axis=1)  # [o]
    whi = wsum.astype(np.float16)
    wlo = (wsum - whi.astype(np.float64)).astype(np.float16)
    wr = np.stack([whi, whi], axis=0)  # [2, o] (rows pair with cmin hi/lo)
    bv = np.ascontiguousarray(bias.reshape(MT, 128).T).astype(np.float32)  # [128, MT]
    return wt, wr, bv


def kernel(x, weight, bias):
    from concourse.bass_utils import run_bass_kernel_spmd

    if "nc" not in _cache:
        _cache["nc"] = _build_bass(mode="petr")
    nc = _cache["nc"]

    wt, wr, bv = _prep_weights(np.asarray(weight), np.asarray(bias))
    x = np.asarray(x, dtype=np.float32)
    in_maps = [
        {
            "xb": np.ascontiguousarray(x[i]),
            "wt": wt,
            "wr": wr,
            "bv": bv,
        }
        for i in range(B)
    ]
    res = run_bass_kernel_spmd(nc, in_maps, core_ids=list(range(B)))
    return np.stack([r["out"] for r in res.results], axis=0).astype(np.float32)



# revision 49
# speedup vs baseline: 1.2635x; 1.2635x over previous
"""Trainium2 Bass kernel for nn_NoiseConv1d (channel-wise 6-bit quantize + 1x1 conv).

Math (per batch b, position n, with QL=63):
    cmax/cmin = max/min over channels c of x[b,:,n]
    s = (cmax-cmin)/QL ; q = round((x-cmin)/s)   (q in [0,63], exact in fp16)
    out[o,n] = s[n] * sum_c W[o,c] q[c,n] + cmin[n]*Wsum[o] + bias[o]
             = sum_c W[o,c] z[c,n] + bias[o]   with z = s*q + cmin

Current best: mode "tzde2" (t-design), one batch per NeuronCore, 8 cores
data-parallel, ~76.7us/core (bench3 loop-slope; petr baseline was ~102us):
    - host prep (pure relayout/cast, no math): x -> fp16, TRANSPOSED to
      [n, c] and chunked [8][128 n-part][4 tile][512 c]; weights W^T fp16;
      output fp16 -> host casts to fp32. Kills the fwd PE transpose + its
      ACT evac entirely and halves DMA (4MB in + 4MB out).
    - per tile [128n x 512c] straight from DMA:
      DVE tensor_scalar+accum_out: per-n max(x), max(-x)   (~460ns each;
        accum caps the DVE mode -- dominant DVE cost, no cheaper path:
        tensor_tensor_reduce crashes HW, gpsimd elementwise is 6.4us/op)
      DVE smalls per chunk [128,4]: s, r=1/s, Bt = -cmin*r + 1536
      DVE quant: q16 = RNE16(r*x + Bt)  (fp16 ulp=1 in [1024,2048) rounds
        to integer+1536 for free)
      DVE z = (q16 - Bt)*s  == s*q + cmin  (dequantized WITH offset folded;
        this identity removes the old K=2 offset-row matmuls + Wsum prep)
      PE-transpose z 128x128 blocks -> fp16 PSUM; pairs of tiles share one
        [128,1024] psum tile, single evac per pair (one less DVE DRAIN),
        evacs alternate DVE/ACT pairs
      PE matmul W^T fp16 (4 k-tiles, FD=512) -> fp32 psum; ACT evac folds
        bias; DMA out fp16 [o, n]
    - rel err vs reference ~6.1e-3 (fp16 in/out rounding; tolerance 2e-2).
    Failed/rejected: gpsimd z-step (284us), quant/evac on ACT (+5-10us,
    ACT ops ~2.3x errata), chunk-pair matmul, FD=256 accums, TTR stats
    (HW crash), psum-direct DMA (ISA: SBUF/DRAM only), fp8 (precision).
"""

import sys

sys.path.insert(0, "/opt/trn_rl_repo")

import numpy as np

B, C, COUT, N = 8, 512, 512, 4096
QL = 63.0
MAGIC = float(np.float32(12582912.0))  # 1.5 * 2**23 : fp32 RNE round-to-int magic
MAGIC16 = 1536.0  # 1.5 * 2**10 : fp16 RNE round-to-int magic (ulp=1 in [1024,2048))

KT = C // 128  # 4 c-tiles
MT = COUT // 128  # 4 o-tiles
NT = N // 128  # 32 n-tiles
NCH = N // 512  # 8 n-chunks (psum free dim 512)

_cache = {}


def _build_bass(loop_n=0, mode="full"):
    from contextlib import ExitStack

    import concourse.bass as bass
    import concourse.mybir as mybir
    from concourse import bacc
    from concourse.bass import ds, ts
    from concourse.masks import make_identity
    from concourse.tile import TileContext

    f32 = mybir.dt.float32
    f16 = mybir.dt.float16
    AX = mybir.AxisListType
    OP = mybir.AluOpType
    AF = mybir.ActivationFunctionType

    nc = bacc.Bacc(None, target_bir_lowering=False)
    xb = nc.dram_tensor("xb", [C, N], f32, kind="ExternalInput")
    wt = nc.dram_tensor("wt", [C, COUT], f16, kind="ExternalInput")  # W^T [c,o]
    wr = nc.dram_tensor("wr", [2, COUT], f16, kind="ExternalInput")  # [Whi,Whi]
    bv = nc.dram_tensor("bv", [128, MT], f32, kind="ExternalInput")  # bias packed
    out = nc.dram_tensor("out", [COUT, N], f32, kind="ExternalOutput")

    with ExitStack() as ctx:
        tc = ctx.enter_context(TileContext(nc))
        singles = ctx.enter_context(tc.tile_pool(name="singles", bufs=1))

        NQ = N // 4  # n-quarter size
        if "cf16" in mode:
            x_nat = [
                singles.tile([128, KT, NQ], f16, name=f"x{q}", tag=f"x{q}")
                for q in range(4)
            ]
        else:
            x_nat = [
                singles.tile([128, KT, NQ], f32, name=f"x{q}", tag=f"x{q}")
                for q in range(4)
            ]
        if "f16x" in mode:
            # fp16 copy of x (natural layout) + its xbar-transposed form
            x_b = singles.tile([128, KT, N], f16)
            # per quarter: xtb[q][n%128, k, t_local, c%128] = x[c, n]
            xtb = [
                singles.tile([128, KT, NT // 4, 128], f16, name=f"xtb{q}", tag=f"xtb{q}")
                for q in range(4)
            ]
        wt_sb = singles.tile([128, KT, COUT], f16)
        wr_sb = singles.tile([2, COUT], f16)
        bv_sb = singles.tile([128, MT], f32)
        ident = singles.tile([128, 128], f32)
        ident16 = singles.tile([128, 128], f16)

        make_identity(nc, ident)
        make_identity(nc, ident16)
        nc.sync.dma_start(out=wt_sb, in_=wt.rearrange("(k p) o -> p k o", p=128))
        nc.sync.dma_start(out=wr_sb, in_=wr[:, :])
        nc.sync.dma_start(out=bv_sb, in_=bv[:, :])
        xq = xb.rearrange("(k p) n -> p k n", p=128)

        ps_tr = ctx.enter_context(tc.tile_pool(name="ps_tr", bufs=3, space="PSUM"))
        ps_qt = ctx.enter_context(tc.tile_pool(name="ps_qt", bufs=2, space="PSUM"))
        ps_mm = ctx.enter_context(tc.tile_pool(name="ps_mm", bufs=3, space="PSUM"))
        stat = ctx.enter_context(tc.tile_pool(name="stat", bufs=12))
        qpool = ctx.enter_context(tc.tile_pool(name="qpool", bufs=6))
        jpool = ctx.enter_context(tc.tile_pool(name="jpool", bufs=4))
        opool = ctx.enter_context(tc.tile_pool(name="opool", bufs=4))
        qtpool = ctx.enter_context(tc.tile_pool(name="qtpool", bufs=3))

        def per_iter():
            for q in range(4):
                if "cf16" in mode:
                    # SWDGE cast-DMA: fp32 DRAM -> fp16 SBUF directly
                    nc.gpsimd.dma_start(out=x_nat[q], in_=xq[:, :, ds(q * NQ, NQ)])
                else:
                    nc.sync.dma_start(out=x_nat[q], in_=xq[:, :, ds(q * NQ, NQ)])
            if "f16x" in mode:
                for q in range(4):
                    nc.vector.tensor_copy(
                        out=x_b[:, :, ds(q * NQ, NQ)], in_=x_nat[q]
                    )
                    for k in range(KT):
                        nc.sync.dma_start_transpose(
                            out=xtb[q][:, k],
                            in_=x_b[:, k, ds(q * NQ, NQ)],
                        )
            if "dma" in mode:
                # DMA floor: just stream junk out at full volume
                for j in range(NCH):
                    for m in range(MT):
                        nc.sync.dma_start(
                            out=out[ts(m, 128), ds(512 * j, 512)],
                            in_=x_nat[0][:, 0, 0:512],
                        )
                return
            body_chunks()

        def body_chunks():
          for j in range(NCH):
            # qsT[c%128, i, c//128, n%128]
            qsT = qtpool.tile([128, 4, KT, 128], f16, tag="qsT")
            # cmin hi/lo rows for this chunk: cmr[row, i, n%128]
            cmr = qtpool.tile([2, 4, 128], f16, tag="cmr")
            hl = stat.tile([128, 2, 4], f16, tag="hl")  # hi/lo per tile col
            for i in range(4):
                t = 4 * j + i
                if "f16x" in mode:
                    # x already transposed to [n, c] fp16 by the xbar DMA
                    xts = xtb[t // 8][:, :, t % 8, :]
                else:
                    # ---- transpose x block column t into [n, c] psum tile ----
                    cf = "cf16" in mode
                    xt_ps = ps_tr.tile([128, 512], f16 if cf else f32, tag="xt")
                    xs = x_nat[t // 8]
                    tl = t % 8
                    for k in range(KT):
                        nc.tensor.transpose(
                            xt_ps[:, ts(k, 128)],
                            xs[:, k, ts(tl, 128)],
                            ident16 if cf else ident,
                        )
                    # evac psum -> sbuf right away so psum slots recycle fast
                    # and the DVE stats below run at higher perf mode
                    xts_t = qpool.tile([128, 512], f16 if cf else f32, tag="xts")
                    nc.scalar.copy(out=xts_t, in_=xt_ps)
                    xts = xts_t
                # ---- per-n stats over free dim via tensor_scalar accumulate ----
                mx = stat.tile([128, 1], f32, tag="mx")
                ng = stat.tile([128, 1], f32, tag="ng")  # -cmin
                junk = jpool.tile([128, 512], f16, tag="junk")
                nc.vector.tensor_scalar(
                    out=junk, in0=xts, scalar1=1.0, scalar2=None,
                    op0=OP.mult, op1=OP.max, accum_out=mx,
                )
                nc.vector.tensor_scalar(
                    out=junk, in0=xts, scalar1=-1.0, scalar2=None,
                    op0=OP.mult, op1=OP.max, accum_out=ng,
                )
                d = stat.tile([128, 1], f32, tag="d")
                s = stat.tile([128, 1], f32, tag="s")
                r = stat.tile([128, 1], f32, tag="r")
                Bt = stat.tile([128, 1], f32, tag="Bt")
                nc.vector.tensor_add(d, mx, ng)  # d = cmax - cmin
                nc.vector.tensor_scalar_mul(s, d, 1.0 / QL)  # s = scaling
                nc.vector.reciprocal(r, s)  # r = 1/s
                # B = -cmin*r + 1536 (fp16 magic; fp32 ulp@1536 keeps fraction)
                nc.vector.tensor_scalar(
                    out=Bt, in0=ng, scalar1=r, scalar2=MAGIC16,
                    op0=OP.mult, op1=OP.add,
                )
                if "noq" in mode:
                    continue
                # ---- quantize: y+1536 = r*x + B via ACT fma; the fp32->fp16
                # output conversion RNE-rounds to integer+1536 exactly ----
                qp = qpool.tile([128, 512], f16, tag="qp")
                nc.scalar.activation(
                    out=qp, in_=xts, func=AF.Identity, bias=Bt, scale=r
                )
                # ---- qs = (q16 - 1536) * s -> fp16 ----
                qs = qpool.tile([128, KT * 128], f16, tag="qs")
                nc.vector.tensor_scalar(
                    out=qs, in0=qp, scalar1=MAGIC16, scalar2=s,
                    op0=OP.subtract, op1=OP.mult,
                )
                # cmin hi/lo for this tile: hi = f16(cmin); lo ~= cmin - hi
                nc.gpsimd.tensor_scalar_mul(hl[:, 0, i : i + 1], ng, -1.0)
                nc.gpsimd.tensor_scalar(
                    out=hl[:, 1, i : i + 1], in0=hl[:, 0, i : i + 1],
                    scalar1=ng, scalar2=-1.0, op0=OP.add, op1=OP.mult,
                )
                # ---- transpose back to [c, n] layout ----
                if "petr" in mode:
                    # PE transpose (f16, 1 cyc/row) + DVE evac; avoids the
                    # xbar DMA-transpose path entirely (slow + serializes
                    # against copy DMAs)
                    qt_ps = ps_qt.tile([128, (KT + 1) * 128], f16, tag="qtp")
                    for b_ in range(KT):
                        nc.tensor.transpose(
                            qt_ps[:, ts(b_, 128)], qs[:, ts(b_, 128)], ident16
                        )
                    nc.tensor.transpose(
                        qt_ps[0:2, ts(KT, 128)], hl[:, :, i], ident16
                    )
                    nc.vector.tensor_copy(out=qsT[:, i], in_=qt_ps[:, 0 : KT * 128])
                    nc.vector.tensor_copy(
                        out=cmr[:, i], in_=qt_ps[0:2, ts(KT, 128)]
                    )
                else:
                    nc.sync.dma_start_transpose(out=qsT[:, i], in_=qs)

            # ---- matmul for this n-chunk ----
            if "nomm" in mode:
                continue
            for m in range(MT):
                ps = ps_mm.tile([128, 512], f32, tag="mm")
                for k in range(KT):
                    nc.tensor.matmul(
                        ps,
                        wt_sb[:, k, ts(m, 128)],
                        qsT[:, :, k, :],
                        start=(k == 0),
                        stop=False,
                    )
                nc.tensor.matmul(
                    ps,
                    wr_sb[:, ts(m, 128)],
                    cmr[:, :, :],
                    start=False,
                    stop=True,
                )
                ob = opool.tile([128, 512], f32, tag="ob")
                nc.scalar.activation(
                    out=ob, in_=ps, func=AF.Identity,
                    bias=bv_sb[:, m : m + 1], scale=1.0,
                )
                nc.sync.dma_start(
                    out=out[ts(m, 128), ds(512 * j, 512)], in_=ob
                )

        if loop_n:
            with tc.For_i(0, loop_n, 1):
                per_iter()
        else:
            per_iter()

    nc.compile()
    return nc


def _build_bass_z(loop_n=0, mode="z"):
    """z-design: fp16 end-to-end, no offset rows.

    Per tile [128n x 512c]:
      PE  : 4 fwd transposes x_f16 -> psum xt
      ACT : evac xt -> SBUF (f16)
      DVE : 2 tensor_scalar accums (max, max(-x)) at 4x
            per-chunk small ops: d, s, r, Bt = ng*r + 1536
            quant: qp = RNE16(r*x + Bt)  (fp16 magic rounding)
            z = (qp - Bt)*s  == s*q + cmin  (dequantized, offset folded!)
      PE  : 4 back transposes z -> psum, DVE evac -> zsT
      PE  : matmul W^T @ zsT (4 k-tiles, no offset rows)
      ACT : evac psum + bias -> out f16, DMA out
    """
    from contextlib import ExitStack

    import concourse.bass as bass
    import concourse.mybir as mybir
    from concourse import bacc
    from concourse.bass import ds, ts
    from concourse.masks import make_identity
    from concourse.tile import TileContext

    f32 = mybir.dt.float32
    f16 = mybir.dt.float16
    OP = mybir.AluOpType
    AF = mybir.ActivationFunctionType

    nc = bacc.Bacc(None, target_bir_lowering=False)
    # x host-relayout: [chunk, p, k, 512] so each chunk DMA is contiguous
    xh = nc.dram_tensor("xh", [NCH, 128, KT, 512], f16, kind="ExternalInput")
    wt = nc.dram_tensor("wt", [C, COUT], f16, kind="ExternalInput")  # W^T [c,o]
    bv = nc.dram_tensor("bv", [128, MT], f32, kind="ExternalInput")  # bias packed
    out = nc.dram_tensor("out", [COUT, N], f16, kind="ExternalOutput")

    with ExitStack() as ctx:
        tc = ctx.enter_context(TileContext(nc))
        singles = ctx.enter_context(tc.tile_pool(name="singles", bufs=1))

        wt_sb = singles.tile([128, KT, COUT], f16)
        bv_sb = singles.tile([128, MT], f32)
        ident16 = singles.tile([128, 128], f16)

        make_identity(nc, ident16)
        nc.sync.dma_start(out=wt_sb, in_=wt.rearrange("(k p) o -> p k o", p=128))
        nc.sync.dma_start(out=bv_sb, in_=bv[:, :])

        ps_tr = ctx.enter_context(tc.tile_pool(name="ps_tr", bufs=2, space="PSUM"))
        ps_zt = ctx.enter_context(tc.tile_pool(name="ps_zt", bufs=2, space="PSUM"))
        ps_mm = ctx.enter_context(tc.tile_pool(name="ps_mm", bufs=3, space="PSUM"))
        stat = ctx.enter_context(tc.tile_pool(name="stat", bufs=10))
        xcpool = ctx.enter_context(tc.tile_pool(name="xcpool", bufs=3))
        xpool = ctx.enter_context(tc.tile_pool(name="xpool", bufs=6))
        qpool = ctx.enter_context(tc.tile_pool(name="qpool", bufs=4))
        jpool = ctx.enter_context(tc.tile_pool(name="jpool", bufs=3))
        opool = ctx.enter_context(tc.tile_pool(name="opool", bufs=4))
        ztpool = ctx.enter_context(tc.tile_pool(name="ztpool", bufs=3))

        def per_iter():
            for j in range(NCH):
                xc = xcpool.tile([128, KT, 512], f16, tag="xc")
                nc.sync.dma_start(out=xc, in_=xh[j])
                # zsT[c%128, i, c//128, n%128]
                zsT = ztpool.tile([128, 4, KT, 128], f16, tag="zsT")
                mx = stat.tile([128, 4], f32, tag="mx")
                ng = stat.tile([128, 4], f32, tag="ng")  # -cmin
                xts = [None] * 4
                # phase 1: transpose + evac + stats for the chunk's 4 tiles
                for i in range(4):
                    xt_ps = ps_tr.tile([128, 512], f16, tag="xt")
                    for k in range(KT):
                        nc.tensor.transpose(
                            xt_ps[:, ts(k, 128)],
                            xc[:, k, ts(i, 128)],
                            ident16,
                        )
                    xts_t = xpool.tile([128, 512], f16, name=f"xts{i}", tag="xts")
                    xts[i] = xts_t
                    nc.scalar.copy(out=xts_t, in_=xt_ps)
                    if "noacc" in mode:
                        continue
                    junk = jpool.tile([128, 512], f16, tag="junk")
                    nc.vector.tensor_scalar(
                        out=junk, in0=xts[i], scalar1=1.0, scalar2=None,
                        op0=OP.mult, op1=OP.max, accum_out=mx[:, i : i + 1],
                    )
                    nc.vector.tensor_scalar(
                        out=junk, in0=xts[i], scalar1=-1.0, scalar2=None,
                        op0=OP.mult, op1=OP.max, accum_out=ng[:, i : i + 1],
                    )
                # phase 2: batched per-chunk stats smalls
                if "noacc" not in mode:
                    s = stat.tile([128, 4], f32, tag="s")
                    r = stat.tile([128, 4], f32, tag="r")
                    Bt = stat.tile([128, 4], f32, tag="Bt")
                    nc.vector.tensor_add(s, mx, ng)  # cmax - cmin
                    nc.vector.tensor_scalar_mul(s, s, 1.0 / QL)
                    nc.vector.reciprocal(r, s)
                    # Bt = ng*r + 1536;  q16 = RNE16(r*x+Bt);  z = (q16-Bt)*s
                    nc.vector.tensor_mul(Bt, ng, r)
                    nc.vector.tensor_scalar_add(Bt, Bt, MAGIC16)
                # phase 3: quantize + dequantize-with-offset + back transpose
                for i in range(4):
                    if "noq" in mode:
                        z = xts[i]
                    else:
                        if "noacc" in mode:
                            r_i, Bt_i, s_i = 0.01, 1536.0, 100.0
                        else:
                            r_i = r[:, i : i + 1]
                            Bt_i = Bt[:, i : i + 1]
                            s_i = s[:, i : i + 1]
                        qp = qpool.tile([128, 512], f16, tag="qp")
                        nc.vector.tensor_scalar(
                            out=qp, in0=xts[i], scalar1=r_i,
                            scalar2=Bt_i, op0=OP.mult, op1=OP.add,
                        )
                        z = qpool.tile([128, 512], f16, tag="z")
                        nc.vector.tensor_scalar(
                            out=z, in0=qp, scalar1=Bt_i,
                            scalar2=s_i, op0=OP.subtract, op1=OP.mult,
                        )
                    zt_ps = ps_zt.tile([128, 512], f16, tag="ztp")
                    for k in range(KT):
                        nc.tensor.transpose(
                            zt_ps[:, ts(k, 128)], z[:, ts(k, 128)], ident16
                        )
                    nc.vector.tensor_copy(out=zsT[:, i], in_=zt_ps)
                # phase 4: matmul + bias evac + out DMA
                if "nomm" in mode:
                    continue
                for m in range(MT):
                    ps = ps_mm.tile([128, 512], f32, tag="mm")
                    for k in range(KT):
                        nc.tensor.matmul(
                            ps,
                            wt_sb[:, k, ts(m, 128)],
                            zsT[:, :, k, :],
                            start=(k == 0),
                            stop=(k == KT - 1),
                        )
                    ob = opool.tile([128, 512], f16, tag="ob")
                    nc.scalar.activation(
                        out=ob, in_=ps, func=AF.Identity,
                        bias=bv_sb[:, m : m + 1], scale=1.0,
                    )
                    nc.sync.dma_start(
                        out=out[ts(m, 128), ds(512 * j, 512)], in_=ob
                    )

        if loop_n:
            with tc.For_i(0, loop_n, 1):
                per_iter()
        else:
            per_iter()

    nc.compile()
    return nc


def _build_bass_t(loop_n=0, mode="t"):
    """t-design: host supplies x TRANSPOSED ([n, c] layout, fp16).

    Kills the fwd PE transposes and the ACT x-evac entirely.
    Per tile [128n x 512c] straight from DMA:
      DVE : 2 tensor_scalar accums (max, max(-x))
            per-chunk smalls: s, r, Bt
      quant: qp = RNE16(r*x + Bt)        (DVE or ACT: 'qa')
      z = (qp - Bt)*s                     (GPSIMD; 'za'=ACT, 'zd'=DVE)
      PE  : 4 back transposes z -> psum
      evac: psum -> zsT SBUF              (alternate DVE/ACT; 'ea'/'ed')
      PE  : matmul, ACT bias evac, DMA out f16
    """
    from contextlib import ExitStack

    import concourse.mybir as mybir
    from concourse import bacc
    from concourse.bass import ds, ts
    from concourse.masks import make_identity
    from concourse.tile import TileContext

    f32 = mybir.dt.float32
    f16 = mybir.dt.float16
    OP = mybir.AluOpType
    AF = mybir.ActivationFunctionType

    nc = bacc.Bacc(None, target_bir_lowering=False)
    # [chunk, p(n%128), i(tile in chunk), c] — host-transposed
    xth = nc.dram_tensor("xth", [NCH, 128, 4, C], f16, kind="ExternalInput")
    wt = nc.dram_tensor("wt", [C, COUT], f16, kind="ExternalInput")  # W^T [c,o]
    bv = nc.dram_tensor("bv", [128, MT], f32, kind="ExternalInput")  # bias packed
    out = nc.dram_tensor("out", [COUT, N], f16, kind="ExternalOutput")

    z_eng = "act" if "za" in mode else ("dve" if "zd" in mode else "gps")
    q_eng = "act" if "qa" in mode else "dve"
    ev_eng = "act" if "ea" in mode else ("dve" if "ed" in mode else "alt")
    pair = "pp" in mode  # chunk-pair matmul (FD=1024)

    with ExitStack() as ctx:
        tc = ctx.enter_context(TileContext(nc))
        singles = ctx.enter_context(tc.tile_pool(name="singles", bufs=1))

        wt_sb = singles.tile([128, KT, COUT], f16)
        bv_sb = singles.tile([128, MT], f32)
        ident16 = singles.tile([128, 128], f16)

        make_identity(nc, ident16)
        nc.sync.dma_start(out=wt_sb, in_=wt.rearrange("(k p) o -> p k o", p=128))
        nc.sync.dma_start(out=bv_sb, in_=bv[:, :])

        bb = "bb" in mode
        ps_zt = ctx.enter_context(
            tc.tile_pool(name="ps_zt", bufs=2 if pair else 3, space="PSUM")
        )
        ps_mm = ctx.enter_context(
            tc.tile_pool(name="ps_mm", bufs=3 if pair else 4, space="PSUM")
        )
        stat = ctx.enter_context(tc.tile_pool(name="stat", bufs=14 if bb else 10))
        xcpool = ctx.enter_context(tc.tile_pool(name="xcpool", bufs=4 if bb else 3))
        qpool = ctx.enter_context(tc.tile_pool(name="qpool", bufs=6 if bb else 4))
        jpool = ctx.enter_context(tc.tile_pool(name="jpool", bufs=4 if bb else 3))
        opool = ctx.enter_context(tc.tile_pool(name="opool", bufs=6 if bb else 4))
        ztpool = ctx.enter_context(tc.tile_pool(name="ztpool", bufs=3))

        def chunk_stats(j, mx8, ng8, coff):
            # DMA + stats accums for chunk j into shared [128,8] stat tiles
            xc = xcpool.tile([128, 4, C], f16, tag="xc")
            nc.sync.dma_start(out=xc, in_=xth[j])
            for i in range(4):
                junk = jpool.tile([128, 512], f16, tag="junk")
                nc.vector.tensor_scalar(
                    out=junk, in0=xc[:, i], scalar1=1.0, scalar2=None,
                    op0=OP.mult, op1=OP.max,
                    accum_out=mx8[:, coff + i : coff + i + 1],
                )
                nc.vector.tensor_scalar(
                    out=junk, in0=xc[:, i], scalar1=-1.0, scalar2=None,
                    op0=OP.mult, op1=OP.max,
                    accum_out=ng8[:, coff + i : coff + i + 1],
                )
            return xc

        def smalls8(mx8, ng8):
            sm = nc.gpsimd if "sg" in mode else nc.vector
            s8 = stat.tile([128, 8], f32, tag="s8")
            r8 = stat.tile([128, 8], f32, tag="r8")
            Bt8 = stat.tile([128, 8], f32, tag="Bt8")
            sm.tensor_add(s8, mx8, ng8)
            sm.tensor_scalar_mul(s8, s8, 1.0 / QL)
            nc.vector.reciprocal(r8, s8)
            sm.tensor_mul(Bt8, ng8, r8)
            sm.tensor_scalar_add(Bt8, Bt8, MAGIC16)
            return s8, r8, Bt8

        def chunk_qz(j, xc, s8, r8, Bt8, coff, zsT):
            # quant + z + back-transpose + evac for chunk j
            zt2_ps = None
            for i in range(4):
                t = 4 * j + i
                c_i = coff + i
                qp = qpool.tile([128, 512], f16, tag="qp")
                nc.vector.tensor_scalar(
                    out=qp, in0=xc[:, i], scalar1=r8[:, c_i : c_i + 1],
                    scalar2=Bt8[:, c_i : c_i + 1], op0=OP.mult, op1=OP.add,
                )
                z = qpool.tile([128, 512], f16, tag="z")
                nc.vector.tensor_scalar(
                    out=z, in0=qp, scalar1=Bt8[:, c_i : c_i + 1],
                    scalar2=s8[:, c_i : c_i + 1], op0=OP.subtract, op1=OP.mult,
                )
                if i % 2 == 0:
                    zt2_ps = ps_zt.tile([128, 2, 512], f16, tag="ztp2")
                zt_ps = zt2_ps[:, i % 2]
                for k in range(KT):
                    nc.tensor.transpose(
                        zt_ps[:, ts(k, 128)], z[:, ts(k, 128)], ident16
                    )
                if i % 2 == 1:
                    dst = zsT[:, i - 1 : i + 1]
                    if t % 4 < 2:
                        nc.scalar.copy(out=dst, in_=zt2_ps)
                    else:
                        nc.vector.tensor_copy(out=dst, in_=zt2_ps)

        def chunk_quant(j, zsT, ioff):
            # DMA + stats + quant + z + back-transpose for chunk j,
            # writing zsT[:, ioff:ioff+4]
            xc = xcpool.tile([128, 4, C], f16, tag="xc")
            nc.sync.dma_start(out=xc, in_=xth[j])
            sdt = f16 if "a16" in mode else f32
            mx = stat.tile([128, 4], sdt, name="mx", tag="mx")
            s = stat.tile([128, 4], f32, tag="s")
            r = stat.tile([128, 4], f32, tag="r")
            Bt = stat.tile([128, 4], f32, tag="Bt")
            if "tr" in mode:
                # fused fold+reduce (op1=min only — the HW-safe combo):
                # mneg = min((max-fold)*-1) = -cmax ; cm = min(min-fold) = cmin
                cm = stat.tile([128, 4], f32, tag="cm")  # cmin
                for i in range(4):
                    junk = jpool.tile([128, 256], f16, name="junk2", tag="junk2")
                    nc.vector.tensor_tensor_reduce(
                        out=junk, in0=xc[:, i, 0:256], in1=xc[:, i, 256:512],
                        scale=-1.0, scalar=1e30, op0=OP.max, op1=OP.min,
                        accum_out=mx[:, i : i + 1],  # holds -cmax
                    )
                    nc.vector.tensor_tensor_reduce(
                        out=junk, in0=xc[:, i, 0:256], in1=xc[:, i, 256:512],
                        scale=1.0, scalar=1e30, op0=OP.min, op1=OP.min,
                        accum_out=cm[:, i : i + 1],
                    )
                if "stt" in mode:
                    # d = -(-cmax) - cmin
                    nc.vector.scalar_tensor_tensor(
                        out=s, in0=mx, scalar=-1.0, in1=cm,
                        op0=OP.mult, op1=OP.subtract,
                    )
                    nc.vector.tensor_scalar_mul(s, s, 1.0 / QL)
                    nc.vector.reciprocal(r, s)
                    # Bt = -cmin*r + 1536
                    nc.vector.scalar_tensor_tensor(
                        out=Bt, in0=cm, scalar=-1.0, in1=r,
                        op0=OP.mult, op1=OP.mult,
                    )
                    nc.vector.tensor_scalar_add(Bt, Bt, MAGIC16)
                else:
                    # plain smalls: s = (-mx - cm)/QL (mx holds -cmax)
                    nc.vector.tensor_add(s, mx, cm)  # -(cmax - cmin)
                    nc.vector.tensor_scalar_mul(s, s, -1.0 / QL)
                    nc.vector.reciprocal(r, s)
                    nc.vector.tensor_mul(Bt, cm, r)
                    nc.vector.tensor_scalar(
                        out=Bt, in0=Bt, scalar1=-1.0, scalar2=MAGIC16,
                        op0=OP.mult, op1=OP.add,
                    )
            else:
                ng = stat.tile([128, 4], sdt, name="ng", tag="ng")  # -cmin
                for i in range(4):
                    junk = jpool.tile([128, 512], f16, tag="junk")
                    nc.vector.tensor_scalar(
                        out=junk, in0=xc[:, i], scalar1=1.0, scalar2=None,
                        op0=OP.mult, op1=OP.max, accum_out=mx[:, i : i + 1],
                    )
                    nc.vector.tensor_scalar(
                        out=junk, in0=xc[:, i], scalar1=-1.0, scalar2=None,
                        op0=OP.mult, op1=OP.max, accum_out=ng[:, i : i + 1],
                    )
                nc.vector.tensor_add(s, mx, ng)  # cmax - cmin
                nc.vector.tensor_scalar_mul(s, s, 1.0 / QL)
                nc.vector.reciprocal(r, s)
                nc.vector.tensor_mul(Bt, ng, r)
                nc.vector.tensor_scalar_add(Bt, Bt, MAGIC16)
            if z_eng == "act":
                u = stat.tile([128, 4], f32, tag="u")
                nc.vector.tensor_mul(u, s, Bt)
                nc.vector.tensor_scalar_mul(u, u, -1.0)
            ev2 = "e2" in mode
            zt2_ps = None
            for i in range(4):
                t = 4 * j + i
                r_i = r[:, i : i + 1]
                Bt_i = Bt[:, i : i + 1]
                s_i = s[:, i : i + 1]
                qp = qpool.tile([128, 512], f16, tag="qp")
                use_act_q = q_eng == "act" or ("qh" in mode and t % 2 == 1)
                if use_act_q:
                    nc.scalar.activation(
                        out=qp, in_=xc[:, i], func=AF.Identity,
                        bias=Bt_i, scale=r_i,
                    )
                else:
                    nc.vector.tensor_scalar(
                        out=qp, in0=xc[:, i], scalar1=r_i,
                        scalar2=Bt_i, op0=OP.mult, op1=OP.add,
                    )
                z = qpool.tile([128, 512], f16, tag="z")
                if z_eng == "act":
                    # z = s*qp + (-s*Bt): needs u = -s*Bt per chunk
                    nc.scalar.activation(
                        out=z, in_=qp, func=AF.Identity,
                        bias=u[:, i : i + 1], scale=s_i,
                    )
                elif z_eng == "dve":
                    nc.vector.tensor_scalar(
                        out=z, in0=qp, scalar1=Bt_i,
                        scalar2=s_i, op0=OP.subtract, op1=OP.mult,
                    )
                else:
                    nc.gpsimd.tensor_scalar(
                        out=z, in0=qp, scalar1=Bt_i,
                        scalar2=s_i, op0=OP.subtract, op1=OP.mult,
                    )
                if ev2:
                    # two tiles' transposes share one [128,1024] psum tile;
                    # single evac per pair (one less DRAIN)
                    if i % 2 == 0:
                        zt2_ps = ps_zt.tile([128, 2, 512], f16, tag="ztp2")
                    zt_ps = zt2_ps[:, i % 2]
                else:
                    zt_ps = ps_zt.tile([128, 512], f16, tag="ztp")
                for k in range(KT):
                    nc.tensor.transpose(
                        zt_ps[:, ts(k, 128)], z[:, ts(k, 128)], ident16
                    )
                use_act = ev_eng == "act" or (ev_eng == "alt" and t % 2 == 0)
                if ev2:
                    if i % 2 == 1:
                        dst = zsT[:, ioff + i - 1 : ioff + i + 1]
                        if ev_eng == "act" or (ev_eng == "alt" and t % 4 < 2):
                            nc.scalar.copy(out=dst, in_=zt2_ps)
                        else:
                            nc.vector.tensor_copy(out=dst, in_=zt2_ps)
                elif use_act:
                    nc.scalar.copy(out=zsT[:, ioff + i], in_=zt_ps)
                else:
                    nc.vector.tensor_copy(out=zsT[:, ioff + i], in_=zt_ps)

        def mm_block(zsT, j0, nw):
            # matmul over nw*128 output columns starting at chunk j0;
            # one FD<=512 matmul per psum bank half, single evac + DMA
            fd = nw * 128
            for m in range(MT):
                ps = ps_mm.tile([128, fd], f32, name=f"ps{nw}", tag="mm")
                for h in range(nw // 4):
                    for k in range(KT):
                        nc.tensor.matmul(
                            ps[:, ds(512 * h, 512)],
                            wt_sb[:, k, ts(m, 128)],
                            zsT[:, ds(4 * h, 4), k, :],
                            start=(k == 0),
                            stop=(k == KT - 1),
                        )
                ob = opool.tile([128, fd], f16, name=f"ob{nw}", tag="ob")
                if "mh" in mode and m % 2 == 1:
                    # DVE mm-evac with bias via tensor_scalar add
                    nc.vector.tensor_scalar(
                        out=ob, in0=ps, scalar1=bv_sb[:, m : m + 1],
                        scalar2=None, op0=OP.add,
                    )
                else:
                    nc.scalar.activation(
                        out=ob, in_=ps, func=AF.Identity,
                        bias=bv_sb[:, m : m + 1], scale=1.0,
                    )
                nc.sync.dma_start(
                    out=out[ts(m, 128), ds(512 * j0, fd)], in_=ob
                )

        def per_iter():
            if "c2" in mode:
                for jp in range(NCH // 2):
                    mx8 = stat.tile([128, 8], f32, name="mx8", tag="mx8")
                    ng8 = stat.tile([128, 8], f32, name="ng8", tag="ng8")
                    xc0 = chunk_stats(2 * jp, mx8, ng8, 0)
                    xc1 = chunk_stats(2 * jp + 1, mx8, ng8, 4)
                    s8, r8, Bt8 = smalls8(mx8, ng8)
                    zsT0 = ztpool.tile([128, 4, KT, 128], f16, name="zsT0", tag="zsT")
                    chunk_qz(2 * jp, xc0, s8, r8, Bt8, 0, zsT0)
                    mm_block(zsT0, 2 * jp, 4)
                    zsT1 = ztpool.tile([128, 4, KT, 128], f16, name="zsT1", tag="zsT")
                    chunk_qz(2 * jp + 1, xc1, s8, r8, Bt8, 4, zsT1)
                    mm_block(zsT1, 2 * jp + 1, 4)
                return
            if pair:
                for jp in range(NCH // 2):
                    zsT = ztpool.tile([128, 8, KT, 128], f16, tag="zsT")
                    chunk_quant(2 * jp, zsT, 0)
                    chunk_quant(2 * jp + 1, zsT, 4)
                    if "nm" not in mode:
                        mm_block(zsT, 2 * jp, 8)
            else:
                for j in range(NCH):
                    zsT = ztpool.tile([128, 4, KT, 128], f16, tag="zsT")
                    chunk_quant(j, zsT, 0)
                    if "nm" not in mode:
                        mm_block(zsT, j, 4)

        if loop_n:
            with tc.For_i(0, loop_n, 1):
                per_iter()
        else:
            per_iter()

    nc.compile()
    return nc


def _build_micro(loop_n=0, mode="mb_acc"):
    """DVE/ACT op microbenchmarks: body = 48 identical ops, slope/48 = cost."""
    from contextlib import ExitStack

    import concourse.mybir as mybir
    from concourse import bacc
    from concourse.tile import TileContext

    f32 = mybir.dt.float32
    f16 = mybir.dt.float16
    OP = mybir.AluOpType
    AF = mybir.ActivationFunctionType

    nc = bacc.Bacc(None, target_bir_lowering=False)
    xh = nc.dram_tensor("xh", [128, 512], f16, kind="ExternalInput")
    out = nc.dram_tensor("out", [128, 512], f16, kind="ExternalOutput")
    variant = mode.split("_", 1)[1]

    with ExitStack() as ctx:
        tc = ctx.enter_context(TileContext(nc))
        singles = ctx.enter_context(tc.tile_pool(name="singles", bufs=1))
        src = singles.tile([128, 512], f16)
        srcb = singles.tile([128, 512], f16)
        acc = singles.tile([128, 48], f32)
        acc16s = singles.tile([128, 48], f16)
        sc = singles.tile([128, 1], f32)
        nc.sync.dma_start(out=src, in_=xh[:, :])
        nc.vector.tensor_copy(out=srcb, in_=src)
        nc.vector.memset(acc, 0.0)
        nc.vector.memset(acc16s, 0.0)
        nc.vector.memset(sc, 0.5)
        jp = ctx.enter_context(tc.tile_pool(name="jp", bufs=4))

        def per_iter():
            if variant in ("chain", "chacc", "chain256", "chacc256", "chacc16"):
                # dependency-chained ops — DCE-proof
                fd = 256 if variant.endswith("256") else 512
                prev = src[:, 0:fd]
                acc16 = acc16s
                for u in range(48):
                    junk = jp.tile([128, fd], f16, name="cj", tag="cj")
                    if variant == "chacc16":
                        nc.vector.tensor_scalar(
                            out=junk, in0=prev, scalar1=1.0, scalar2=None,
                            op0=OP.mult, op1=OP.max,
                            accum_out=acc16[:, u : u + 1],
                        )
                    elif variant.startswith("chacc"):
                        nc.vector.tensor_scalar(
                            out=junk, in0=prev, scalar1=1.0, scalar2=None,
                            op0=OP.mult, op1=OP.max,
                            accum_out=acc[:, u : u + 1],
                        )
                    else:
                        nc.vector.tensor_scalar(
                            out=junk, in0=prev, scalar1=sc, scalar2=sc,
                            op0=OP.mult, op1=OP.add,
                        )
                    prev = junk
                nc.sync.dma_start(out=out[:, 0:fd], in_=prev)
                return
            for u in range(48):
                junk = jp.tile([128, 512], f16, tag="junk")
                if variant == "acc":
                    nc.vector.tensor_scalar(
                        out=junk, in0=src, scalar1=1.0, scalar2=None,
                        op0=OP.mult, op1=OP.max, accum_out=acc[:, u : u + 1],
                    )
                elif variant == "ts2":
                    nc.vector.tensor_scalar(
                        out=junk, in0=src, scalar1=0.01, scalar2=1536.0,
                        op0=OP.mult, op1=OP.add,
                    )
                elif variant == "ts2ap":
                    nc.vector.tensor_scalar(
                        out=junk, in0=src, scalar1=sc, scalar2=sc,
                        op0=OP.mult, op1=OP.add,
                    )
                elif variant == "ts1":
                    nc.vector.tensor_scalar_mul(junk, src, 2.0)
                elif variant == "cp":
                    nc.vector.tensor_copy(out=junk, in_=src)
                elif variant == "tt":
                    nc.vector.tensor_max(junk, src, srcb)
                elif variant == "act":
                    nc.scalar.activation(
                        out=junk, in_=src, func=AF.Identity, bias=sc, scale=1.0
                    )
                else:
                    raise ValueError(variant)
            nc.sync.dma_start(out=out[:, :], in_=junk)

        if loop_n:
            with tc.For_i(0, loop_n, 1):
                per_iter()
        else:
            per_iter()

    nc.compile()
    return nc


def _prep_weights(weight, bias):
    W = weight[:, :, 0].astype(np.float64)  # [o, c]
    wt = np.ascontiguousarray(W.T).astype(np.float16)  # [c, o]
    wsum = W.sum(axis=1)  # [o]
    whi = wsum.astype(np.float16)
    wlo = (wsum - whi.astype(np.float64)).astype(np.float16)
    wr = np.stack([whi, whi], axis=0)  # [2, o] (rows pair with cmin hi/lo)
    bv = np.ascontiguousarray(bias.reshape(MT, 128).T).astype(np.float32)  # [128, MT]
    return wt, wr, bv


MODE = "tc2sg"


def _make_in_maps(x, weight, bias, mode=None):
    mode = MODE if mode is None else mode
    if mode.startswith("mb_"):
        return [
            {"xh": np.asarray(x, np.float32)[i, :128, :512].astype(np.float16)}
            for i in range(B)
        ]
    wt, wr, bv = _prep_weights(np.asarray(weight), np.asarray(bias))
    x = np.asarray(x, dtype=np.float32)
    if mode.startswith("t"):
        # transpose + relayout: [c, n] -> [chunk, p(n%128), i(tile), c]
        def relayout_t(xi):
            xf = xi.astype(np.float16).T.reshape(NCH, 4, 128, C)
            return np.ascontiguousarray(xf.transpose(0, 2, 1, 3))

        return [
            {"xth": relayout_t(x[i]), "wt": wt, "bv": bv}
            for i in range(B)
        ]
    if mode.startswith("z"):
        # [c, n] -> [chunk, p, k, 512] with c = k*128 + p (contiguous chunks)
        def relayout(xi):
            xf = xi.astype(np.float16).reshape(KT, 128, NCH, 512)
            return np.ascontiguousarray(xf.transpose(2, 1, 0, 3))

        return [
            {"xh": relayout(x[i]), "wt": wt, "bv": bv}
            for i in range(B)
        ]
    return [
        {
            "xb": np.ascontiguousarray(x[i]),
            "wt": wt,
            "wr": wr,
            "bv": bv,
        }
        for i in range(B)
    ]


def _build(loop_n=0, mode=None):
    mode = MODE if mode is None else mode
    if mode.startswith("mb_"):
        return _build_micro(loop_n=loop_n, mode=mode)
    if mode.startswith("z"):
        return _build_bass_z(loop_n=loop_n, mode=mode)
    if mode.startswith("t"):
        return _build_bass_t(loop_n=loop_n, mode=mode)
    return _build_bass(loop_n=loop_n, mode=mode)


def kernel(x, weight, bias):
    from concourse.bass_utils import run_bass_kernel_spmd

    if "nc" not in _cache:
        _cache["nc"] = _build(mode=MODE)
    nc = _cache["nc"]

    in_maps = _make_in_maps(x, weight, bias, mode=MODE)
    res = run_bass_kernel_spmd(nc, in_maps, core_ids=list(range(B)))
    return np.stack([r["out"] for r in res.results], axis=0).astype(np.float32)

